# revision 1
# baseline (speedup 1.0000x reference)
"""DLRM forward (26-table EmbeddingBag + dot interaction + MLPs) on 8 trn2 cores.

Strategy: batch-parallel across the 8 cores (2048 samples each), embedding
tables replicated in each core's HBM so no collectives are needed.

Per-core pipeline (all on device):
  bottom MLP (feature-major, PE) -> xbot [64, BC]
  gather: SWDGE indirect DMA from flat emb table [T*V, 64]; bag-sum pooling
          is done in-DMA via compute_op=add chains (4 gathers per group)
  pooled [samples, t, d] -> PE transposes -> arr [64, (q, s, i)] per 128-sample
          tile, where col block q holds the 4 samples' 27 T-vectors (i=0 is x)
  syrk: gram_q = arr_q^T arr_q -> diagonal 32x32 blocks hold Z per sample
  G2 (diag-block extract, strided DVE copies) -> PE transposes -> G2T with
          layout [27*qq + j, (k, s, i)]  (sample = 4*(4k+qq)+s)
  tril-contraction: Z never extracted per-sample; instead contracted straight
          against top-MLP layer-0 weights grouped by pair row index i
          (lhsT = W_i constant), accumulated in PSUM with the x-part matmul
  top MLP layers 1/2 + sigmoid -> out [4, BC/4] (qq-blocks; host unpermutes)
"""

import numpy as np
from contextlib import ExitStack

import concourse.bass as bass
import concourse.bacc as bacc
import concourse.tile as tile
from concourse import mybir
from concourse.bass_utils import run_bass_kernel_spmd
from concourse.masks import make_identity

F32 = mybir.dt.float32
BF16 = mybir.dt.bfloat16
I32 = mybir.dt.int32
AF = mybir.ActivationFunctionType
ALU = mybir.AluOpType

B = 16384
L = 4
D = 64
T = 26
TT = T + 1  # 27
V = 200000
NCORES = 8
BC = B // NCORES  # 2048 per core


def build_program(bc=BC, v=V, gt=1):
    """Build the single-core SPMD Bass program. bc must be a multiple of 128*gt."""
    ntile = bc // 128           # 128-sample tiles
    ng = ntile // gt            # gather groups (gt tiles each)
    kc = bc // 16               # number of k (q-quads)
    ne = bc // 4                # cols per qq output block
    nb = min(512, bc)           # matmul N chunk
    nn = bc // nb

    nc = bacc.Bacc()

    def din(name, shape, dtype=F32):
        return nc.declare_dram_parameter(name, shape, dtype, isOutput=False)

    x_in = din("x_in", [13, bc])
    emb = din("emb", [T * v, D])
    gidx = din("gidx", [ng * 128, gt * T * L], I32)
    wb0t = din("wb0t", [13, 512])
    bb0 = din("bb0", [128, 4])
    wb1t = din("wb1t", [128, 4 * 256])
    bb1 = din("bb1", [128, 2])
    wb2t = din("wb2t", [128, 2 * 64])
    bb2 = din("bb2", [64, 1])
    wt0x = din("wt0x", [64, 512])
    wall = din("wall", [128, T * 512], BF16)
    bt0 = din("bt0", [128, 4])
    wt1t = din("wt1t", [128, 4 * 256])
    bt1 = din("bt1", [128, 2])
    wt2t = din("wt2t", [128, 2])
    bt2 = din("bt2", [1, 1])
    out = nc.declare_dram_parameter("out", [4, ne], F32, isOutput=True)

    with TileCtx(nc) as tc, ExitStack() as ctx:
        cpool = ctx.enter_context(tc.tile_pool(name="const", bufs=1))
        psA = ctx.enter_context(tc.tile_pool(name="psA", bufs=3, space="PSUM"))
        psB = ctx.enter_context(tc.tile_pool(name="psB", bufs=4, space="PSUM"))
        psC = ctx.enter_context(tc.tile_pool(name="psC", bufs=1, space="PSUM"))
        work = ctx.enter_context(tc.tile_pool(name="work", bufs=1))
        pooled_p = ctx.enter_context(tc.tile_pool(name="pooled", bufs=2))
        idx_p = ctx.enter_context(tc.tile_pool(name="idx", bufs=4))
        arr_p = ctx.enter_context(tc.tile_pool(name="arr", bufs=2))
        gram_p = ctx.enter_context(tc.tile_pool(name="gram", bufs=2))
        g2_p = ctx.enter_context(tc.tile_pool(name="g2", bufs=2))
        act_p = ctx.enter_context(tc.tile_pool(name="act", bufs=4))

        def load(dram, shape, dtype=F32):
            t = cpool.tile(shape, dtype, tag=f"c_{dram.name}")
            nc.sync.dma_start(out=t[:], in_=dram[:])
            return t

        ident = cpool.tile([128, 128], F32)
        make_identity(nc, ident[:])
        wb0t_t = load(wb0t, [13, 512])
        bb0_t = load(bb0, [128, 4])
        wb1t_t = load(wb1t, [128, 4 * 256])
        bb1_t = load(bb1, [128, 2])
        wb2t_t = load(wb2t, [128, 2 * 64])
        bb2_t = load(bb2, [64, 1])
        wt0x_t = cpool.tile([128, 512], F32, tag="c_wt0x")
        nc.sync.dma_start(out=wt0x_t[0:64, :], in_=wt0x[:])
        wall_t = load(wall, [128, T * 512], BF16)
        bt0_t = load(bt0, [128, 4])
        wt1t_t = load(wt1t, [128, 4 * 256])
        bt1_t = load(bt1, [128, 2])
        wt2t_t = load(wt2t, [128, 2])
        bt2_t = load(bt2, [1, 1])

        # ---------------- bottom MLP (feature-major, column-blocked) ----------
        xbot_t = work.tile([128, bc], F32)
        xbot = xbot_t[0:64, :]
        xin_t = work.tile([13, bc], F32)
        nc.sync.dma_start(out=xin_t[:], in_=x_in[:])
        with tc.tile_pool(name="bot", bufs=2) as bot_p:
            for n in range(nn):
                y0n = bot_p.tile([128, 4 * nb], F32, tag="y0n")
                for m in range(4):
                    ps = psB.tile([128, nb], F32)
                    nc.tensor.matmul(
                        ps[:],
                        lhsT=wb0t_t[:, m * 128:(m + 1) * 128],
                        rhs=xin_t[:, n * nb:(n + 1) * nb],
                        start=True, stop=True,
                    )
                    nc.scalar.activation(
                        y0n[:, m * nb:(m + 1) * nb],
                        ps[:], AF.Relu, bias=bb0_t[:, m:m + 1],
                    )
                y1n = bot_p.tile([128, 2 * nb], F32, tag="y1n")
                for m in range(2):
                    ps = psB.tile([128, nb], F32)
                    for k in range(4):
                        nc.tensor.matmul(
                            ps[:],
                            lhsT=wb1t_t[:, k * 256 + m * 128: k * 256 + (m + 1) * 128],
                            rhs=y0n[:, k * nb:(k + 1) * nb],
                            start=(k == 0), stop=(k == 3),
                        )
                    nc.scalar.activation(
                        y1n[:, m * nb:(m + 1) * nb],
                        ps[:], AF.Relu, bias=bb1_t[:, m:m + 1],
                    )
                ps = psB.tile([128, nb], F32)
                for k in range(2):
                    nc.tensor.matmul(
                        ps[:64, :],
                        lhsT=wb2t_t[:, k * 64:(k + 1) * 64],
                        rhs=y1n[:, k * nb:(k + 1) * nb],
                        start=(k == 0), stop=(k == 1),
                    )
                nc.scalar.activation(
                    xbot[:, n * nb:(n + 1) * nb], ps[:64, :], AF.Relu,
                    bias=bb2_t[:, 0:1],
                )
        xbv = xbot.rearrange("d (k r s) -> d k r s", r=4, s=4)

        # ---------------- gather + pool + interaction prep ----------------
        # two G2T tiles: A holds qq 0 (rows 0..26) and qq 1 (rows 32..58),
        # B holds qq 2 / qq 3 likewise -- matmul bases stay in {0, 32}
        g2ta = work.tile([64, kc * 128], BF16)
        g2tb = work.tile([64, kc * 128], BF16)
        for g in range(ng):
            pooled = pooled_p.tile([128, gt * T * D], F32)
            pview = pooled[:].rearrange("p (c d) -> p c d", d=D)
            it = idx_p.tile([128, gt * T * L], I32)
            nc.sync.dma_start(
                out=it[:], in_=gidx[g * 128:(g + 1) * 128, :]
            )
            # HW indirect DMA supports exactly one offset per dest partition,
            # so each (table, bag-elem) is its own 128-row gather; the bag sum
            # is accumulated in-DMA via the CCE add op.
            for c in range(gt * T):
                for l in range(4):
                    nc.gpsimd.indirect_dma_start(
                        out=pview[:, c, :],
                        out_offset=None,
                        in_=emb[:],
                        in_offset=bass.IndirectOffsetOnAxis(
                            ap=it[:, c * L + l:c * L + l + 1], axis=0),
                        compute_op=(ALU.bypass if l == 0 else ALU.add),
                    )
            for u in range(gt):
                tg = g * gt + u  # global tile == arr chunk
                arr_c = arr_p.tile([64, 32 * 128], BF16)
                arr_v = arr_c[:].rearrange("d (q s i) -> d q s i", s=4, i=32)
                # zero the pad cols i=27..31 (syrk reads full 128-col blocks)
                nc.vector.memset(arr_v[:, :, :, 27:32], 0.0)
                for up in range(13):
                    pst = psA.tile([128, 128], F32, tag="t128")
                    nc.tensor.transpose(
                        pst[:],
                        pooled[:, (u * T + 2 * up) * D:(u * T + 2 * up + 2) * D],
                        ident[:],
                    )
                    for h in range(2):
                        nc.any.tensor_copy(
                            out=arr_v[:, :, :, 1 + 2 * up + h],
                            in_=pst[h * 64:(h + 1) * 64, :].rearrange(
                                "d (q s) -> d q s", s=4),
                        )
                nc.vector.tensor_copy(
                    out=arr_v[:, :, :, 0],
                    in_=xbot_t[0:64, tg * 128:(tg + 1) * 128].rearrange(
                        "d (q s) -> d q s", s=4),
                )
                g2_c = g2_p.tile([128, 32 * 32], F32)
                nc.vector.memset(g2_c[:], 0.0)
                for half in range(4):
                    gram_c = gram_p.tile([128, 8 * 128], F32)
                    for ql in range(8):
                        q_loc = half * 8 + ql
                        psg = psA.tile([128, 128], F32, tag="t128")
                        nc.tensor.matmul(
                            psg[:],
                            lhsT=arr_c[:, q_loc * 128:(q_loc + 1) * 128],
                            rhs=arr_c[:, q_loc * 128:(q_loc + 1) * 128],
                            start=True, stop=True,
                        )
                        nc.any.tensor_copy(
                            out=gram_c[:, ql * 128:(ql + 1) * 128], in_=psg[:]
                        )
                    for s in range(4):
                        nc.vector.tensor_copy(
                            out=g2_c[32 * s:32 * s + 27, :].rearrange(
                                "i (q j) -> i q j", j=32)[
                                :, half * 8:(half + 1) * 8, 0:27],
                            in_=gram_c[32 * s:32 * s + 27, :].rearrange(
                                "i (q c) -> i q c", c=128)[:, :, 32 * s:32 * s + 27],
                        )
                for w in range(8):
                    k = tg * 8 + w
                    psta = psA.tile([64, 128], F32, tag="t128")
                    nc.tensor.transpose(
                        psta[:], g2_c[:, w * 128:w * 128 + 64], ident[:]
                    )
                    nc.any.tensor_copy(
                        out=g2ta[:, k * 128:(k + 1) * 128], in_=psta[:]
                    )
                    pstb = psA.tile([64, 128], F32, tag="t128")
                    nc.tensor.transpose(
                        pstb[:], g2_c[:, w * 128 + 64:(w + 1) * 128], ident[:]
                    )
                    nc.any.tensor_copy(
                        out=g2tb[:, k * 128:(k + 1) * 128], in_=pstb[:]
                    )

        # ---------------- tril-contraction + top MLP ----------------
        for qq in range(4):
            g2half = g2ta if qq < 2 else g2tb
            hb = 32 * (qq % 2)
            g2s = g2half[hb:hb + 27, :].rearrange(
                "j (k s i) -> j k s i", s=4, i=32)
            y0q = []
            for m in range(4):
                ps = psB.tile([128, ne], F32)
                for i in range(1, TT):
                    nc.tensor.matmul(
                        ps[:],
                        lhsT=wall_t[hb:hb + 27,
                                    (i - 1) * 512 + m * 128:
                                    (i - 1) * 512 + (m + 1) * 128],
                        rhs=g2s[:, :, :, i],
                        start=(i == 1), stop=False,
                    )
                nc.tensor.matmul(
                    ps[:],
                    lhsT=wt0x_t[0:64, m * 128:(m + 1) * 128],
                    rhs=xbv[:, :, qq, :],
                    start=False, stop=True,
                )
                t0 = act_p.tile([128, ne], F32, tag="yq")
                nc.scalar.activation(t0[:], ps[:], AF.Relu, bias=bt0_t[:, m:m + 1])
                y0q.append(t0)
            y1q = []
            for m in range(2):
                ps = psB.tile([128, ne], F32)
                for k in range(4):
                    nc.tensor.matmul(
                        ps[:],
                        lhsT=wt1t_t[:, k * 256 + m * 128: k * 256 + (m + 1) * 128],
                        rhs=y0q[k][:],
                        start=(k == 0), stop=(k == 3),
                    )
                t1 = act_p.tile([128, ne], F32, tag="yq")
                nc.scalar.activation(t1[:], ps[:], AF.Relu, bias=bt1_t[:, m:m + 1])
                y1q.append(t1)
            ps2 = psC.tile([1, ne], F32)
            for k in range(2):
                nc.tensor.matmul(
                    ps2[:],
                    lhsT=wt2t_t[:, k:k + 1],
                    rhs=y1q[k][:],
                    start=(k == 0), stop=(k == 1),
                )
            ot = act_p.tile([1, ne], F32, tag="yq")
            nc.scalar.activation(ot[:], ps2[:], AF.Sigmoid, bias=bt2_t[:, 0:1])
            nc.sync.dma_start(out=out[qq:qq + 1, :], in_=ot[:])

    nc.finalize()
    return nc


def TileCtx(nc):
    return tile.TileContext(nc)


# ---------------------------------------------------------------------------
# host-side packing
# ---------------------------------------------------------------------------

def pack_weights(ws):
    """ws: dict of reference weight arrays -> dict of packed f32 arrays."""
    f = lambda a: np.ascontiguousarray(a, dtype=np.float32)
    o = {}
    o["wb0t"] = f(ws["Wb0"].T)                         # [13, 512]
    o["bb0"] = f(ws["bb0"].reshape(4, 128).T)          # [128, 4]
    w1 = ws["Wb1"].T                                   # [512, 256]
    o["wb1t"] = f(np.concatenate([w1[128 * k:128 * (k + 1)] for k in range(4)], 1))
    o["bb1"] = f(ws["bb1"].reshape(2, 128).T)
    w2 = ws["Wb2"].T                                   # [256, 64]
    o["wb2t"] = f(np.concatenate([w2[128 * k:128 * (k + 1)] for k in range(2)], 1))
    o["bb2"] = f(ws["bb2"].reshape(64, 1))
    wt0 = np.asarray(ws["Wt0"], dtype=np.float64)      # [512, 415]
    o["wt0x"] = f(wt0[:, :64].T)                       # [64, 512]
    wall = np.zeros((128, T * 512), dtype=np.float32)
    for i in range(1, TT):
        off = i * (i - 1) // 2
        # W_i[j, m] = Wt0[m, 64 + off + j] for j < i; replicated in all
        # four 32-row bands so lhsT base always matches rhs base 32*qq
        for qq in range(4):
            wall[32 * qq:32 * qq + i, (i - 1) * 512:i * 512] = \
                wt0[:, 64 + off:64 + off + i].T
    import ml_dtypes
    o["wall"] = wall.astype(ml_dtypes.bfloat16)
    o["bt0"] = f(ws["bt0"].reshape(4, 128).T)
    t1 = ws["Wt1"].T                                   # [512, 256]
    o["wt1t"] = f(np.concatenate([t1[128 * k:128 * (k + 1)] for k in range(4)], 1))
    o["bt1"] = f(ws["bt1"].reshape(2, 128).T)
    t2 = ws["Wt2"].T                                   # [256, 1]
    o["wt2t"] = f(np.concatenate([t2[128 * k:128 * (k + 1)] for k in range(2)], 1))
    o["bt2"] = f(ws["bt2"].reshape(1, 1))
    return o


def pack_gidx(lsi_core, bc, v, gt=1):
    """lsi_core: [T, bc*L] int indices for this core's samples.

    returns [ng*128, gt*T*L] int32 with entry [g*128+p, (u*T+t)*L + l]
      = t*v + lsi_core[t, (128*(g*gt+u)+p)*L + l]
    """
    ntile = bc // 128
    ng = ntile // gt
    li = np.asarray(lsi_core).reshape(T, bc, L)        # [t, n, l]
    li = li.reshape(T, ng, gt, 128, L)                 # [t, g, u, p, l]
    gi = li + (np.arange(T, dtype=np.int64) * v)[:, None, None, None, None]
    gi = gi.transpose(1, 3, 2, 0, 4)                   # [g, p, u, t, l]
    return np.ascontiguousarray(
        gi.reshape(ng * 128, gt * T * L), dtype=np.int32)


def unpermute_out(out_c, bc):
    """out_c: [4, bc/4] (qq, 4k+s) -> [bc] in natural sample order l'=16k+4qq+s."""
    kc = bc // 16
    o = out_c.reshape(4, kc, 4)        # [qq, k, s]
    o = o.transpose(1, 0, 2)           # [k, qq, s]
    return np.ascontiguousarray(o.reshape(bc))


_PROG = None
TRACE = False          # set by test harness to collect an NTFF profile
TRACE_KW = {}
LAST_RESULTS = None    # BassKernelResults of the most recent run


def kernel(**inputs):
    global _PROG, LAST_RESULTS
    dense_x = np.asarray(inputs["dense_x"], dtype=np.float32)     # [B, 13]
    lsi = np.asarray(inputs["lS_i"]).reshape(T, B * L)
    emb = np.asarray(inputs["emb"], dtype=np.float32)             # [T, V, D]
    emb_flat = np.ascontiguousarray(emb.reshape(T * V, D))

    ws = {k: np.asarray(inputs[k]) for k in (
        "Wb0", "bb0", "Wb1", "bb1", "Wb2", "bb2",
        "Wt0", "bt0", "Wt1", "bt1", "Wt2", "bt2")}
    packed_w = pack_weights(ws)

    if _PROG is None:
        _PROG = build_program()
    nc = _PROG

    lsi_r = lsi.reshape(T, B, L)
    in_maps = []
    for c in range(NCORES):
        sl = slice(BC * c, BC * (c + 1))
        m = dict(packed_w)
        m["x_in"] = np.ascontiguousarray(dense_x[sl].T)           # [13, BC]
        m["gidx"] = pack_gidx(lsi_r[:, sl, :].reshape(T, BC * L), BC, V)
        m["emb"] = emb_flat
        in_maps.append(m)

    bkr = run_bass_kernel_spmd(
        nc, in_maps, list(range(NCORES)), trace=TRACE, **TRACE_KW)
    LAST_RESULTS = bkr
    outs = [unpermute_out(np.asarray(r["out"]), BC) for r in bkr.results]
    return np.concatenate(outs).reshape(B, 1).astype(np.float32)



# revision 8
# speedup vs baseline: 7.1542x; 7.1542x over previous
"""DLRM forward (26-table EmbeddingBag + dot interaction + MLPs) on 8 trn2 cores.

Strategy: batch-parallel across the 8 cores (2048 samples each), embedding
tables replicated (bf16) in each core's HBM so no collectives are needed.

Per-core pipeline (all on device):
  bottom MLP (feature-major, PE, bf16) -> xbot [64, BC]
  gather: batched SWDGE indirect DMA from flat bf16 emb table [T*V, 64];
          one op per (group, bag-slot l) carrying gt*T offsets per partition;
          bag-sum pooling is done in-DMA via CCE add chains
  pooled [128, gt*T*64] -> PE transposes -> arr [64, (q, s, i)] per 128-sample
          tile (q = 8h + w blocks of 4 samples, i = Tcat row, i=0 is x)
  gram: psG_q = arr_q^T arr_q in PSUM; diagonal 32x32 blocks hold Z per sample
  G2T4R: strided PSUM->SBUF copies exploit gram symmetry to build
          [32a + j, (g, h, tg, w, s)] bf16 with i = 2g + a + 1 -- two 27-row
          j-bands stacked so the interaction contraction runs 59 deep
  top L0: per (h, m): 13 accumulating 54-deep matmuls (lhsT = wall2 chunk)
          + one 64-deep x-part matmul; then top L1/L2 + sigmoid ->
          out [4, BC/4] (h-blocks; host unpermutes)
"""

import numpy as np
from contextlib import ExitStack

import concourse.bass as bass
import concourse.bacc as bacc
import concourse.tile as tile
from concourse import mybir
from concourse.bass_utils import run_bass_kernel_spmd
from concourse.masks import make_identity

F32 = mybir.dt.float32
BF16 = mybir.dt.bfloat16
I32 = mybir.dt.int32
AF = mybir.ActivationFunctionType
ALU = mybir.AluOpType

B = 16384
L = 4
D = 64
T = 26
TT = T + 1  # 27
V = 200000
NCORES = 8
BC = B // NCORES  # 2048 per core
NG2 = 13          # i-pair groups: i = 2g + a + 1, g in [0,13), a in {0,1}


def build_program(bc=BC, v=V, gt=4, cc=None):
    """Build the single-core SPMD Bass program. bc must be a multiple of 128*gt."""
    ntile = bc // 128           # 128-sample tiles
    ng = ntile // gt            # gather groups (gt tiles each)
    ne = bc // 4                # cols per h output block == cols per (g, h)
    nb = min(512, bc)           # matmul N chunk
    nn = bc // nb
    C = gt * T                  # gathered rows per partition per l-slot
    if cc is None:
        cc = C                  # offsets per indirect DMA op (<= C)

    nc = bacc.Bacc()

    def din(name, shape, dtype=BF16):
        return nc.declare_dram_parameter(name, shape, dtype, isOutput=False)

    x_in = din("x_in", [13, bc])
    emb = din("emb", [T * v, D])
    gidx = din("gidx", [ng * 128, L * C], I32)
    wb0t = din("wb0t", [13, 512])
    bb0 = din("bb0", [128, 4], F32)
    wb1t = din("wb1t", [128, 4 * 256])
    bb1 = din("bb1", [128, 2], F32)
    wb2t = din("wb2t", [128, 2 * 64])
    bb2 = din("bb2", [64, 1], F32)
    wt0x = din("wt0x", [64, 512])
    wall2 = din("wall2", [64, NG2 * 512])
    bt0 = din("bt0", [128, 4], F32)
    wt1t = din("wt1t", [128, 4 * 256])
    bt1 = din("bt1", [128, 2], F32)
    wt2t = din("wt2t", [128, 2])
    bt2 = din("bt2", [1, 1], F32)
    out = nc.declare_dram_parameter("out", [4, ne], F32, isOutput=True)

    with tile.TileContext(nc) as tc, ExitStack() as ctx:
        cpool = ctx.enter_context(tc.tile_pool(name="const", bufs=1))
        psT = ctx.enter_context(tc.tile_pool(name="psT", bufs=2, space="PSUM"))
        psG = ctx.enter_context(tc.tile_pool(name="psG", bufs=2, space="PSUM"))
        psB = ctx.enter_context(tc.tile_pool(name="psB", bufs=2, space="PSUM"))
        work = ctx.enter_context(tc.tile_pool(name="work", bufs=1))
        pooled_p = ctx.enter_context(tc.tile_pool(name="pooled", bufs=2))
        idx_p = ctx.enter_context(tc.tile_pool(name="idx", bufs=2))
        arr_p = ctx.enter_context(tc.tile_pool(name="arr", bufs=2))
        act_p = ctx.enter_context(tc.tile_pool(name="act", bufs=4))

        def load(dram, shape, dtype=BF16):
            t = cpool.tile(shape, dtype, tag=f"c_{dram.name}")
            nc.sync.dma_start(out=t[:], in_=dram[:])
            return t

        ident = cpool.tile([128, 128], BF16)
        make_identity(nc, ident[:])
        wb0t_t = load(wb0t, [13, 512])
        bb0_t = load(bb0, [128, 4], F32)
        wb1t_t = load(wb1t, [128, 4 * 256])
        bb1_t = load(bb1, [128, 2], F32)
        wb2t_t = load(wb2t, [128, 2 * 64])
        bb2_t = load(bb2, [64, 1], F32)
        wt0x_t = load(wt0x, [64, 512])
        wall2_t = load(wall2, [64, NG2 * 512])
        bt0_t = load(bt0, [128, 4], F32)
        wt1t_t = load(wt1t, [128, 4 * 256])
        bt1_t = load(bt1, [128, 2], F32)
        wt2t_t = load(wt2t, [128, 2])
        bt2_t = load(bt2, [1, 1], F32)

        # round-robin copy engine assignment (both can read PSUM)
        cp_state = [0]

        def copy(out_ap, in_ap):
            cp_state[0] += 1
            if cp_state[0] % 2:
                nc.vector.tensor_copy(out=out_ap, in_=in_ap)
            else:
                nc.scalar.activation(out_ap, in_ap, AF.Copy)

        # ---------------- bottom MLP (feature-major, column-blocked) ----------
        xbot_t = work.tile([64, bc], BF16)
        xbot = xbot_t[:]
        xin_t = work.tile([13, bc], BF16)
        nc.sync.dma_start(out=xin_t[:], in_=x_in[:])
        with tc.tile_pool(name="bot", bufs=2) as bot_p:
            for n in range(nn):
                y0n = bot_p.tile([128, 4 * nb], BF16, tag="y0n")
                for m in range(4):
                    ps = psB.tile([128, nb], F32, tag="ps")
                    nc.tensor.matmul(
                        ps[:],
                        lhsT=wb0t_t[:, m * 128:(m + 1) * 128],
                        rhs=xin_t[:, n * nb:(n + 1) * nb],
                        start=True, stop=True,
                    )
                    nc.scalar.activation(
                        y0n[:, m * nb:(m + 1) * nb],
                        ps[:], AF.Relu, bias=bb0_t[:, m:m + 1],
                    )
                y1n = bot_p.tile([128, 2 * nb], BF16, tag="y1n")
                for m in range(2):
                    ps = psB.tile([128, nb], F32, tag="ps")
                    for k in range(4):
                        nc.tensor.matmul(
                            ps[:],
                            lhsT=wb1t_t[:, k * 256 + m * 128: k * 256 + (m + 1) * 128],
                            rhs=y0n[:, k * nb:(k + 1) * nb],
                            start=(k == 0), stop=(k == 3),
                        )
                    nc.scalar.activation(
                        y1n[:, m * nb:(m + 1) * nb],
                        ps[:], AF.Relu, bias=bb1_t[:, m:m + 1],
                    )
                ps = psB.tile([128, nb], F32, tag="ps")
                for k in range(2):
                    nc.tensor.matmul(
                        ps[:64, :],
                        lhsT=wb2t_t[:, k * 64:(k + 1) * 64],
                        rhs=y1n[:, k * nb:(k + 1) * nb],
                        start=(k == 0), stop=(k == 1),
                    )
                nc.scalar.activation(
                    xbot[:, n * nb:(n + 1) * nb], ps[:64, :], AF.Relu,
                    bias=bb2_t[:, 0:1],
                )

        # ---------------- gather + pool + interaction prep ----------------
        # G2T4R[32a + j, (((g*4 + h)*ntile + tg)*32 + w*4 + s)] = Z[i, j] of
        # sample (tg, q=8h+w, s), i = 2g + a + 1.  Built by strided copies
        # straight out of gram PSUM using gram symmetry.
        g2t4r = work.tile([64, NG2 * 4 * ntile * 32], BF16)
        # bands live at partition rows 32a + j (32-aligned starts); rows
        # 27..31 are dead -- zero once so the 0:59-deep contraction is clean
        nc.gpsimd.memset(g2t4r[:], 0.0)
        for g in range(ng):
            pooled = pooled_p.tile([128, C * D], BF16)
            it = idx_p.tile([128, L * C], I32)
            nc.sync.dma_start(
                out=it[:], in_=gidx[g * 128:(g + 1) * 128, :]
            )
            # batched indirect gathers per bag slot (cc offsets per op);
            # bag sum accumulates in-DMA via the CCE add op
            for l in range(L):
                for c0 in range(0, C, cc):
                    c1 = min(C, c0 + cc)
                    nc.gpsimd.indirect_dma_start(
                        out=pooled[:, c0 * D:c1 * D],
                        out_offset=None,
                        in_=emb[:],
                        in_offset=bass.IndirectOffsetOnAxis(
                            ap=it[:, l * C + c0:l * C + c1], axis=0),
                        compute_op=(ALU.bypass if l == 0 else ALU.add),
                    )
            for u in range(gt):
                tg = g * gt + u  # global tile index
                arr_c = arr_p.tile([64, 32 * 128], BF16)
                arr_v = arr_c[:].rearrange("d (q s i) -> d q s i", s=4, i=32)
                # zero the pad cols i=27..31 (gram reads full 128-col blocks)
                nc.gpsimd.memset(arr_v[:, :, :, 27:32], 0.0)
                # transpose pooled into arr; 4 transposes batched per PSUM tile
                for b in range(4):
                    ntr = min(4, 13 - 4 * b)
                    pst = psT.tile([128, 512], BF16, tag="pst")
                    for tr in range(ntr):
                        up = 4 * b + tr
                        nc.tensor.transpose(
                            pst[:, tr * 128:(tr + 1) * 128],
                            pooled[:, (u * T + 2 * up) * D:(u * T + 2 * up + 2) * D],
                            ident[:],
                        )
                    for h2 in range(2):
                        # psT band h2 holds tables t = 2(4b+tr) + h2, which
                        # land at arr i = t + 1 = 8b + 2tr + h2 + 1
                        i0 = 8 * b + h2 + 1
                        copy(
                            arr_v[:, :, :, i0:i0 + 2 * ntr:2].transpose(
                                [0, 3, 1, 2]),
                            pst[h2 * 64:(h2 + 1) * 64, 0:ntr * 128].rearrange(
                                "d (t q s) -> d t q s", t=ntr, s=4),
                        )
                # i=0 is x from the bottom MLP
                copy(
                    arr_v[:, :, :, 0],
                    xbot[:, tg * 128:(tg + 1) * 128].rearrange(
                        "d (q s) -> d q s", s=4),
                )
                g2t4r_v = g2t4r[:].rearrange(
                    "j (gg hh kk ww ss) -> j gg hh kk ww ss",
                    gg=NG2, hh=4, kk=ntile, ss=4)
                for h in range(4):
                    psg = psG.tile([128, 8 * 128], F32, tag="psg")
                    for ql in range(8):
                        q = 8 * h + ql
                        nc.tensor.matmul(
                            psg[:, ql * 128:(ql + 1) * 128],
                            lhsT=arr_c[:, q * 128:(q + 1) * 128],
                            rhs=arr_c[:, q * 128:(q + 1) * 128],
                            start=True, stop=True,
                        )
                    psg_v = psg[:].rearrange("p (q c) -> p q c", c=128)
                    for s in range(4):
                        for a in range(2):
                            # src: psg[32s + j, q*128 + 32s + i], i = 2g+a+1
                            #   (gram symmetry: this is Z[i, j] of sample
                            #    (tg, q=8h+ql, s))
                            # dst: g2t4r[32a + j, (g, h, tg, w=ql, s)]
                            copy(
                                g2t4r_v[32 * a:32 * a + 27, :, h, tg, :, s]
                                .transpose([0, 2, 1]),
                                psg_v[32 * s:32 * s + 27, :,
                                      32 * s + 1 + a:32 * s + 27:2],
                            )

        # ---------------- stacked tril-contraction + top MLP ----------------
        xbv = xbot.rearrange("d (k hh w s) -> d k hh w s", hh=4, w=8, s=4)
        for h in range(4):
            y0q = []
            for m in range(4):
                ps = psB.tile([128, ne], F32, tag="ps")
                for gg in range(NG2):
                    nc.tensor.matmul(
                        ps[:],
                        lhsT=wall2_t[0:59, gg * 512 + m * 128:
                                     gg * 512 + (m + 1) * 128],
                        rhs=g2t4r[0:59, (gg * 4 + h) * ne:(gg * 4 + h + 1) * ne],
                        start=(gg == 0), stop=False,
                    )
                nc.tensor.matmul(
                    ps[:],
                    lhsT=wt0x_t[0:64, m * 128:(m + 1) * 128],
                    rhs=xbv[:, :, h, :, :],
                    start=False, stop=True,
                )
                t0 = act_p.tile([128, ne], BF16, tag="yq")
                nc.scalar.activation(t0[:], ps[:], AF.Relu, bias=bt0_t[:, m:m + 1])
                y0q.append(t0)
            y1q = []
            for m in range(2):
                ps = psB.tile([128, ne], F32, tag="ps")
                for k in range(4):
                    nc.tensor.matmul(
                        ps[:],
                        lhsT=wt1t_t[:, k * 256 + m * 128: k * 256 + (m + 1) * 128],
                        rhs=y0q[k][:],
                        start=(k == 0), stop=(k == 3),
                    )
                t1 = act_p.tile([128, ne], BF16, tag="yq")
                nc.scalar.activation(t1[:], ps[:], AF.Relu, bias=bt1_t[:, m:m + 1])
                y1q.append(t1)
            ps2 = psB.tile([128, ne], F32, tag="ps")
            for k in range(2):
                nc.tensor.matmul(
                    ps2[0:1, :],
                    lhsT=wt2t_t[:, k:k + 1],
                    rhs=y1q[k][:],
                    start=(k == 0), stop=(k == 1),
                )
            ot = act_p.tile([1, ne], F32, tag="ot")
            nc.scalar.activation(ot[:], ps2[0:1, :], AF.Sigmoid, bias=bt2_t[:, 0:1])
            nc.sync.dma_start(out=out[h:h + 1, :], in_=ot[:])

    nc.finalize()
    return nc


# ---------------------------------------------------------------------------
# host-side packing
# ---------------------------------------------------------------------------

def _bf(a):
    import ml_dtypes
    return np.ascontiguousarray(np.asarray(a, dtype=np.float32)).astype(
        ml_dtypes.bfloat16)


def pack_weights(ws):
    """ws: dict of reference weight arrays -> dict of packed arrays."""
    f = lambda a: np.ascontiguousarray(a, dtype=np.float32)
    o = {}
    o["wb0t"] = _bf(ws["Wb0"].T)                       # [13, 512]
    o["bb0"] = f(ws["bb0"].reshape(4, 128).T)          # [128, 4]
    w1 = np.asarray(ws["Wb1"], dtype=np.float32).T     # [512, 256]
    o["wb1t"] = _bf(np.concatenate([w1[128 * k:128 * (k + 1)] for k in range(4)], 1))
    o["bb1"] = f(ws["bb1"].reshape(2, 128).T)
    w2 = np.asarray(ws["Wb2"], dtype=np.float32).T     # [256, 64]
    o["wb2t"] = _bf(np.concatenate([w2[128 * k:128 * (k + 1)] for k in range(2)], 1))
    o["bb2"] = f(ws["bb2"].reshape(64, 1))
    wt0 = np.asarray(ws["Wt0"], dtype=np.float64)      # [512, 415]
    o["wt0x"] = _bf(wt0[:, :64].T)                     # [64, 512]
    # wall2[32a + j, g*512 + m] = Wt0[m, 64 + off(i) + j], i = 2g + a + 1,
    # zero for j >= i (rows 27..31 and 59..63 are dead)
    wall2 = np.zeros((64, NG2 * 512), dtype=np.float32)
    for g in range(NG2):
        for a in range(2):
            i = 2 * g + a + 1
            off = i * (i - 1) // 2
            wall2[32 * a:32 * a + i, g * 512:(g + 1) * 512] = \
                wt0[:, 64 + off:64 + off + i].T
    o["wall2"] = _bf(wall2)
    o["bt0"] = f(ws["bt0"].reshape(4, 128).T)
    t1 = np.asarray(ws["Wt1"], dtype=np.float32).T     # [512, 256]
    o["wt1t"] = _bf(np.concatenate([t1[128 * k:128 * (k + 1)] for k in range(4)], 1))
    o["bt1"] = f(ws["bt1"].reshape(2, 128).T)
    t2 = np.asarray(ws["Wt2"], dtype=np.float32).T     # [256, 1]
    o["wt2t"] = _bf(np.concatenate([t2[128 * k:128 * (k + 1)] for k in range(2)], 1))
    o["bt2"] = f(ws["bt2"].reshape(1, 1))
    return o


def pack_gidx(lsi_core, bc, v, gt=4):
    """lsi_core: [T, bc*L] int indices for this core's samples.

    returns [ng*128, L*gt*T] int32 with entry [g*128+p, (l*gt + u)*T + t]
      = t*v + lsi_core[t, (128*(g*gt+u)+p)*L + l]
    """
    ntile = bc // 128
    ng = ntile // gt
    li = np.asarray(lsi_core).reshape(T, bc, L)        # [t, n, l]
    li = li.reshape(T, ng, gt, 128, L)                 # [t, g, u, p, l]
    gi = li + (np.arange(T, dtype=np.int64) * v)[:, None, None, None, None]
    gi = gi.transpose(1, 3, 4, 2, 0)                   # [g, p, l, u, t]
    return np.ascontiguousarray(
        gi.reshape(ng * 128, L * gt * T), dtype=np.int32)


def unpermute_out(out_c, bc):
    """out_c: [4, bc/4] (h, tg*32 + w*4 + s) -> [bc] natural sample order
    (sample = tg*128 + 32h + 4w + s)."""
    ntile = bc // 128
    o = out_c.reshape(4, ntile, 32)    # [h, tg, ws]
    o = o.transpose(1, 0, 2)           # [tg, h, ws]
    return np.ascontiguousarray(o.reshape(bc))


_PROG = None
TRACE = False          # set by test harness to collect an NTFF profile
TRACE_KW = {}
LAST_RESULTS = None    # BassKernelResults of the most recent run


def kernel(**inputs):
    global _PROG, LAST_RESULTS
    dense_x = np.asarray(inputs["dense_x"], dtype=np.float32)     # [B, 13]
    lsi = np.asarray(inputs["lS_i"]).reshape(T, B * L)
    emb = np.asarray(inputs["emb"], dtype=np.float32)             # [T, V, D]
    emb_flat = _bf(emb.reshape(T * V, D))

    ws = {k: np.asarray(inputs[k]) for k in (
        "Wb0", "bb0", "Wb1", "bb1", "Wb2", "bb2",
        "Wt0", "bt0", "Wt1", "bt1", "Wt2", "bt2")}
    packed_w = pack_weights(ws)

    if _PROG is None:
        import os
        _PROG = build_program(
            gt=int(os.environ.get("KGT", "4")),
            cc=(int(os.environ["KCC"]) if "KCC" in os.environ else None))
    nc = _PROG

    lsi_r = lsi.reshape(T, B, L)
    in_maps = []
    for c in range(NCORES):
        sl = slice(BC * c, BC * (c + 1))
        m = dict(packed_w)
        m["x_in"] = _bf(dense_x[sl].T)                            # [13, BC]
        m["gidx"] = pack_gidx(lsi_r[:, sl, :].reshape(T, BC * L), BC, V)
        m["emb"] = emb_flat
        in_maps.append(m)

    bkr = run_bass_kernel_spmd(
        nc, in_maps, list(range(NCORES)), trace=TRACE, **TRACE_KW)
    LAST_RESULTS = bkr
    outs = [unpermute_out(np.asarray(r["out"]), BC) for r in bkr.results]
    return np.concatenate(outs).reshape(B, 1).astype(np.float32)
